# revision 15
# baseline (speedup 1.0000x reference)
"""Trainium2 Bass kernel for a dense GQA transformer block (B=1, T=2048, C=2048,
16 q heads / 8 kv heads, hs=128, SwiGLU FFN=5632), SPMD across 8 NeuronCores.

Sharding: tensor-parallel attention (2 q heads + 1 kv head per core, full T),
one AllToAll per local head to re-shard from head-parallel to row-parallel,
then the attn projection, residual, norm2 and the whole MLP run row-parallel
(256 rows/core, full weights streamed from HBM as bf16).

Key scheduling ideas vs a straightforward version:
- Lazy rms-norm 1: qkv matmuls run on RAW x (so they start as soon as x is
  resident); the per-token 1/rms scale r is computed concurrently (squares on
  the Act engine, column sums via ones-matmuls) and folded into the RoPE
  cos/sin tables for q/k and into the PSUM->SBUF copy of v (tensor_scalar).
- Attention: softmax denominators accumulate on the PE via per-block
  ones-matmuls into PSUM (no DVE adds on the critical path); exp runs on Act;
  causal masking is a DVE multiply with a precomputed mask.
- The two per-head AllToAlls are hidden behind attention head 1 and behind a
  split attn-projection (first accumulate head-0's 8 y-blocks into 16 open
  PSUM co-tiles, then head-1's 8 blocks when its A2A lands).

All activations stay feature-major [C, T]/[HS, T]; matmuls map directly onto
the PE; partition-dim reductions/broadcasts use ones matmuls. bf16 inputs to
the PE with fp32 PSUM accumulation.
"""

import numpy as np
import ml_dtypes

N_CORES = 8
T = 2048
C = 2048
NH = 16
NKV = 8
HS = 128
FFN = 5632
EPS = 1e-5
R = T // N_CORES          # 256 rows (tokens) per core after the A2A
NCB = C // 128            # 16 feature blocks
NFB = FFN // 128          # 44 FFN blocks
NTCH = T // 512           # 4 T-chunks of 512
SM_SCALE = 1.0 / np.sqrt(np.float32(HS))
SM_BIAS = -10.0           # softmax exp bias; max |score| measured ~7, f32 exp safe
BF16 = ml_dtypes.bfloat16

_CACHE = {}


def _build():
    import concourse.mybir as mybir
    import concourse.tile as tile
    from concourse import bacc

    f32 = mybir.dt.float32
    bf16 = mybir.dt.bfloat16
    Exp = mybir.ActivationFunctionType.Exp
    Silu = mybir.ActivationFunctionType.Silu
    Sqrt = mybir.ActivationFunctionType.Sqrt
    Square = mybir.ActivationFunctionType.Square

    nc = bacc.Bacc(trn_type="TRN2", num_devices=N_CORES)

    # ---- kernel I/O (all host-pre-arranged to partition-major layouts) ----
    xP = nc.dram_tensor("xP", [128, NCB * T], bf16, kind="ExternalInput")
    cosT = nc.dram_tensor("cosT", [128, T], bf16, kind="ExternalInput")
    sinT = nc.dram_tensor("sinT", [128, T], bf16, kind="ExternalInput")
    # qkv weight tiles: [p, (db*16+cb)*128+f], db: 0=q0 1=q1 2=k 3=v
    wqkv = nc.dram_tensor("wqkv", [128, 64 * 128], bf16, kind="ExternalInput")
    # attn proj tiles per cout block: [co][p, yb*128+f]
    wproj = nc.dram_tensor("wproj", [16, 128, 16 * 128], bf16, kind="ExternalInput")
    # fc1|fc2 tiles per FFN block: [fb][p, (s*16+cb)*128+f]
    w12 = nc.dram_tensor("w12", [NFB, 128, 2 * 16 * 128], bf16, kind="ExternalInput")
    # mlp proj tiles per cout block: [co][p, fb*128+f]
    w3 = nc.dram_tensor("w3", [16, 128, NFB * 128], bf16, kind="ExternalInput")
    # residual x rows (this core's R tokens), c-major: [p, co*R+t]
    xrows = nc.dram_tensor("xrows", [128, 16 * R], f32, kind="ExternalInput")
    outT = nc.dram_tensor("outT", [C, R], f32, kind="ExternalOutput")

    with tile.TileContext(nc) as tc:
        with (
            tc.tile_pool(name="const", bufs=1) as constp,
            tc.tile_pool(name="dram", bufs=1, space="DRAM") as dramp,
            tc.tile_pool(name="w12s", bufs=7) as w12p,
            tc.tile_pool(name="qkv_acts", bufs=1) as qvp,
        ):
            # ---------------- constants ----------------
            ones_col = constp.tile([128, 1], bf16)
            nc.vector.memset(ones_col, 1.0)
            inv128_col = constp.tile([128, 1], f32)
            nc.vector.memset(inv128_col, 1.0 / 128.0)
            ones_row = constp.tile([1, 128], f32)
            nc.vector.memset(ones_row, 1.0)
            eps_t = constp.tile([128, 1], f32)
            nc.vector.memset(eps_t, EPS)
            smbias_t = constp.tile([128, 1], f32)
            nc.vector.memset(smbias_t, SM_BIAS)
            masks = constp.tile([128, 4 * 512], bf16)
            nc.vector.memset(masks, 1.0)
            for j in range(4):
                # keep 1 where tq >= tk + 128*j  (iota = -x + y - 128j >= 0)
                nc.gpsimd.affine_select(
                    out=masks[:, j * 512:(j + 1) * 512],
                    in_=masks[:, j * 512:(j + 1) * 512],
                    compare_op=mybir.AluOpType.is_ge,
                    fill=0.0,
                    base=-128 * j,
                    pattern=[[1, 512]],
                    channel_multiplier=-1,
                )

            # a2a buffers (one collective per local head, fired as each
            # head's attention completes -> hides trigger latency + core skew)
            a2a_in0 = dramp.tile([8 * 128, R], bf16)
            a2a_out0 = dramp.tile([8 * 128, R], bf16)
            a2a_in1 = dramp.tile([8 * 128, R], bf16)
            a2a_out1 = dramp.tile([8 * 128, R], bf16)

            qk_sb = qvp.tile([128, 3 * T], bf16)     # roped+scaled q0|q1|k, d-major
            v_sb = qvp.tile([128, NCB * 128], bf16)  # scaled v token-major tiles

            with (
                tc.tile_pool(name="cs", bufs=1) as csp,
                tc.tile_pool(name="wqp", bufs=1) as wqpool,
                tc.tile_pool(name="xbfp", bufs=1) as xbfp,
            ):
                cs_sb = csp.tile([128, 2 * T], bf16)
                rcs = csp.tile([128, 2 * T], bf16)   # r-scaled cos|sin tables
                r_col = csp.tile([128, 16], f32)     # r as columns (v scaling)
                wq_sb = wqpool.tile([128, 64 * 128], bf16)
                xsb = xbfp.tile([128, NCB * T], bf16)

                # ============ phase 1: x load + lazy rms-norm stats ==========
                with (
                    nc.named_scope("norm1"),
                    tc.tile_pool(name="xsq", bufs=3) as sqp,
                    tc.tile_pool(name="n1small", bufs=1) as n1s,
                    tc.tile_pool(name="ps_ss", bufs=1, space="PSUM") as pss,
                    tc.tile_pool(name="ps_bc", bufs=2, space="PSUM") as psb,
                    tc.tile_pool(name="ps_rc", bufs=1, space="PSUM") as psrc,
                ):
                    ss_ps = [pss.tile([1, 512], f32, name=f"ss{t4}", tag=f"ss{t4}")
                             for t4 in range(NTCH)]
                    for cb in range(NCB):
                        xsl = xsb[:, cb * T:(cb + 1) * T]
                        nc.sync.dma_start(xsl, xP[:, cb * T:(cb + 1) * T])
                        xsq = sqp.tile([128, T], bf16, tag="xsq")
                        nc.scalar.activation(xsq[:], xsl, Square)
                        for t4 in range(NTCH):
                            nc.tensor.matmul(
                                ss_ps[t4][:], ones_col[:],
                                xsq[:, t4 * 512:(t4 + 1) * 512],
                                start=(cb == 0), stop=(cb == NCB - 1))
                    # qkv weights + rope tables load behind the x stream
                    for db in range(4):
                        nc.sync.dma_start(wq_sb[:, db * 2048:(db + 1) * 2048],
                                          wqkv[:, db * 2048:(db + 1) * 2048])
                    nc.scalar.dma_start(cs_sb[:, 0:T], cosT[:])
                    nc.scalar.dma_start(cs_sb[:, T:2 * T], sinT[:])

                    # r = 1/sqrt(mean(x^2)+eps): sg=sqrt(...), bcast, recip
                    sg = n1s.tile([1, T], f32)
                    rbc = n1s.tile([128, T], f32)
                    for t4 in range(NTCH):
                        ch = slice(t4 * 512, (t4 + 1) * 512)
                        nc.scalar.activation(sg[:, ch], ss_ps[t4][:],
                                             Sqrt, bias=eps_t[0:1, :], scale=1.0 / C)
                        bc = psb.tile([128, 512], f32, tag="bc")
                        nc.tensor.matmul(bc[:], ones_row[:], sg[:, ch],
                                         start=True, stop=True)
                        nc.vector.reciprocal_approx_fast(out=rbc[:, ch], in_=bc[:])
                    # r-scaled rope tables (fold norm into q/k path)
                    nc.vector.tensor_mul(rcs[:, 0:T], cs_sb[:, 0:T], rbc[:])
                    nc.vector.tensor_mul(rcs[:, T:2 * T], cs_sb[:, T:2 * T], rbc[:])
                    # r as token-major columns (for v scaling): transpose the
                    # broadcast tile via (rbc_tile)^T @ (ones/128)
                    rc_ps = psrc.tile([128, 16], f32, tag="rcol")
                    for tb in range(16):
                        nc.tensor.matmul(rc_ps[:, tb:tb + 1],
                                         rbc[:, tb * 128:(tb + 1) * 128],
                                         inv128_col[:], start=True, stop=True)
                    nc.vector.tensor_copy(r_col[:], rc_ps[:])

                cosl = rcs[0:64, 0:T]
                cosh = rcs[64:128, 0:T]
                sinl = rcs[0:64, T:2 * T]
                sinh = rcs[64:128, T:2 * T]

                # ================= phase 2: qkv + rope =================
                with (
                    nc.named_scope("qkv"),
                    tc.tile_pool(name="ropetmp", bufs=6) as rtp,
                    tc.tile_pool(name="ps_qk", bufs=4, space="PSUM") as psqk,
                    tc.tile_pool(name="ps_v", bufs=3, space="PSUM") as psv,
                ):
                    for db in range(3):  # q0, q1, k -> d-major, rope applies r
                        for t4 in range(NTCH):
                            qp = psqk.tile([128, 512], f32, tag="qk")
                            for cb in range(NCB):
                                nc.tensor.matmul(
                                    qp[:],
                                    wq_sb[:, (db * 16 + cb) * 128:
                                          (db * 16 + cb + 1) * 128],
                                    xsb[:, cb * T + t4 * 512: cb * T + (t4 + 1) * 512],
                                    start=(cb == 0), stop=(cb == NCB - 1))
                            # rope into qk_sb. SBUF-SBUF DVE ops need equal base
                            # partitions; PSUM inputs are exempt, so crossed-half
                            # terms read q straight from PSUM.
                            ch = slice(t4 * 512, (t4 + 1) * 512)
                            dst = qk_sb[:, db * T + t4 * 512: db * T + (t4 + 1) * 512]
                            rc = rtp.tile([128, 512], bf16, tag="rc")
                            nc.vector.tensor_mul(rc[:], qp[:], rcs[:, 0:T][:, ch])
                            cross = rtp.tile([128, 512], bf16, tag="cross")
                            nc.vector.tensor_mul(cross[0:64, :], qp[64:128, :],
                                                 sinl[:, ch])
                            nc.vector.tensor_mul(cross[64:128, :], qp[0:64, :],
                                                 sinh[:, ch])
                            nc.vector.tensor_sub(dst[0:64, :], rc[0:64, :],
                                                 cross[0:64, :])
                            nc.vector.tensor_add(dst[64:128, :], rc[64:128, :],
                                                 cross[64:128, :])
                    for tb_ in range(NCB):  # v token-major, r applied on copy-out
                        vp = psv.tile([128, 128], f32, tag="v")
                        for cb in range(NCB):
                            nc.tensor.matmul(
                                vp[:],
                                xsb[:, cb * T + tb_ * 128: cb * T + (tb_ + 1) * 128],
                                wq_sb[:, (48 + cb) * 128:(48 + cb + 1) * 128],
                                start=(cb == 0), stop=(cb == NCB - 1))
                        nc.vector.tensor_scalar_mul(
                            v_sb[:, tb_ * 128:(tb_ + 1) * 128], vp[:],
                            r_col[:, tb_:tb_ + 1])

            # x / cos / qkv-weight buffers freed here; MLP+proj weights stream in.
            # allocate all MLP fc weight tiles now so their DMAs start during attn
            w12_tiles = {}
            for fb in range(NFB):
                w12_tiles[fb] = w12p.tile(
                    [128, 2 * 16 * 128], bf16, name=f"w12t{fb}", tag="w12t")
                nc.sync.dma_start(w12_tiles[fb][:], w12[fb])

            with (
                tc.tile_pool(name="wprojs", bufs=6) as projp,
                tc.tile_pool(name="late", bufs=1) as latep,
            ):
                proj_tiles = []
                for co in range(16):
                    wt = projp.tile([128, 16 * 128], bf16, name=f"projw{co}",
                                    tag="projw")
                    nc.sync.dma_start(wt[:], wproj[co])
                    proj_tiles.append(wt)

                x2_sb = latep.tile([128, 16 * R], f32)
                xn2_sb = latep.tile([128, 16 * R], bf16)
                h_sb = latep.tile([128, NFB * R], bf16)
                y_all = latep.tile([128, 16 * R], bf16)

                # ============ phase 3: attention (2 heads per core) ============
                with (
                    nc.named_scope("attn"),
                    tc.tile_pool(name="pp_p", bufs=7) as ppool,
                    tc.tile_pool(name="pp_y", bufs=4) as ypool,
                    tc.tile_pool(name="attn_small", bufs=4) as asml,
                    tc.tile_pool(name="ps_s", bufs=4, space="PSUM") as ps_s,
                    tc.tile_pool(name="ps_y", bufs=2, space="PSUM") as ps_y,
                    tc.tile_pool(name="ps_sum", bufs=1, space="PSUM") as ps_sum,
                    tc.tile_pool(name="ps_abc", bufs=1, space="PSUM") as ps_abc,
                ):
                    for h in range(2):
                        a2a_in_h = a2a_in0 if h == 0 else a2a_in1
                        q_ap = qk_sb[:, h * T:(h + 1) * T]
                        k_ap = qk_sb[:, 2 * T:3 * T]
                        for qi in range(NTCH):
                            nkb = 4 * qi + 4
                            yp = ps_y.tile([128, 512], f32, tag="y")
                            sump = ps_sum.tile([1, 512], f32, tag="sum")
                            pend = []  # SW pipeline: AV + sum-MM trail scores
                            ptiles = {}

                            def flush(kb):
                                ppt = ptiles.pop(kb)
                                nc.tensor.matmul(
                                    yp[:], v_sb[:, kb * 128:(kb + 1) * 128],
                                    ppt[:], start=(kb == 0), stop=(kb == nkb - 1))
                                nc.tensor.matmul(
                                    sump[:], ones_col[:], ppt[:],
                                    start=(kb == 0), stop=(kb == nkb - 1))

                            for kb in range(nkb):
                                sp = ps_s.tile([128, 512], f32, tag="s")
                                nc.tensor.matmul(
                                    sp[:], k_ap[:, kb * 128:(kb + 1) * 128],
                                    q_ap[:, qi * 512:(qi + 1) * 512],
                                    start=True, stop=True)
                                pt = ppool.tile([128, 512], bf16, tag="p")
                                nc.scalar.activation(pt[:], sp[:], Exp,
                                                     bias=smbias_t[:],
                                                     scale=float(SM_SCALE))
                                if kb >= 4 * qi:
                                    moff = kb - 4 * qi
                                    nc.vector.tensor_mul(
                                        pt[:], pt[:],
                                        masks[:, moff * 512:(moff + 1) * 512])
                                ptiles[kb] = pt
                                pend.append(kb)
                                if len(pend) > 3:
                                    flush(pend.pop(0))
                            while pend:
                                flush(pend.pop(0))
                            # 1/S broadcast, normalize, stage into A2A buffer
                            ssb = asml.tile([1, 512], f32, tag="ssb")
                            nc.scalar.copy(ssb[:], sump[:])
                            bcp = ps_abc.tile([128, 512], f32, tag="abc")
                            nc.tensor.matmul(bcp[:], ones_row[:], ssb[:],
                                             start=True, stop=True)
                            bsb = asml.tile([128, 512], f32, tag="bsb")
                            nc.vector.reciprocal_approx_fast(out=bsb[:], in_=bcp[:])
                            ysb = ypool.tile([128, 512], bf16, tag="ysb")
                            nc.vector.tensor_mul(ysb[:], yp[:], bsb[:])
                            # scatter two 256-token halves to this head's A2A buf
                            for half in range(2):
                                g = 2 * qi + half
                                nc.sync.dma_start(
                                    a2a_in_h[128 * g: 128 * (g + 1), :],
                                    ysb[:, half * 256:(half + 1) * 256])
                        # fire this head's A2A as soon as its outputs are staged,
                        # and pull the result into SBUF immediately
                        a2a_out_h = a2a_out0 if h == 0 else a2a_out1
                        nc.gpsimd.collective_compute(
                            "AllToAll", mybir.AluOpType.bypass,
                            replica_groups=[list(range(N_CORES))],
                            ins=[a2a_in_h.opt()], outs=[a2a_out_h.opt()])
                        for g in range(8):
                            eng = nc.sync if h == 0 else (
                                nc.sync, nc.scalar, nc.gpsimd)[g % 3]
                            eng.dma_start(
                                y_all[:, (h * 8 + g) * R:(h * 8 + g + 1) * R],
                                a2a_out_h[g * 128:(g + 1) * 128, :])

                # ==== phase 5: proj (split over the two A2As) + norm2 ====
                xq2s = []
                with (
                    nc.named_scope("proj"),
                    tc.tile_pool(name="xrow", bufs=1) as xrp,
                    tc.tile_pool(name="ps_acc", bufs=1, space="PSUM") as psa,
                ):
                    # 8 co accumulators = 8 PSUM banks (one accumulation group
                    # per bank: start=True clears has_written BANK-wide, so
                    # groups must never share a bank). The first 8 co split
                    # their yb accumulation across the two A2As; co 8-15 ring-
                    # reuse the banks afterwards.
                    xr_sb = xrp.tile([128, 16 * R], f32)
                    nc.scalar.dma_start(xr_sb[:], xrows[:])
                    aps = [psa.tile([128, R], f32, name=f"acc{j}", tag=f"acc{j}")
                           for j in range(8)]

                    def close_co(co, ap):
                        cs_ = slice(co * R, (co + 1) * R)
                        nc.vector.tensor_add(x2_sb[:, cs_], ap[:], xr_sb[:, cs_])
                        xq2 = latep.tile([128, R], bf16, name=f"xq2_{co}")
                        nc.vector.tensor_mul(xq2[:], x2_sb[:, cs_], x2_sb[:, cs_])
                        xq2s.append(xq2)

                    for yh in range(2):
                        for j in range(8):
                            wt = proj_tiles[j]
                            for yb in range(8 * yh, 8 * yh + 8):
                                nc.tensor.matmul(
                                    aps[j][:], wt[:, yb * 128:(yb + 1) * 128],
                                    y_all[:, yb * R:(yb + 1) * R],
                                    start=(yb == 0), stop=(yb == 15))
                    for j in range(8):
                        close_co(j, aps[j])
                    for j in range(8):
                        co = 8 + j
                        ap = psa.tile([128, R], f32, name=f"accb{j}", tag=f"acc{j}")
                        wt = proj_tiles[co]
                        for yb in range(16):
                            nc.tensor.matmul(
                                ap[:], wt[:, yb * 128:(yb + 1) * 128],
                                y_all[:, yb * R:(yb + 1) * R],
                                start=(yb == 0), stop=(yb == 15))
                        close_co(co, ap)

                with (
                    nc.named_scope("norm2"),
                    tc.tile_pool(name="n2small", bufs=1) as n2s,
                    tc.tile_pool(name="ps_ss2", bufs=1, space="PSUM") as pss2,
                    tc.tile_pool(name="ps_bc2", bufs=1, space="PSUM") as psb2,
                ):
                    ss2 = pss2.tile([1, R], f32, tag="ss2")
                    for co in range(16):
                        nc.tensor.matmul(ss2[:], ones_col[:], xq2s[co][:],
                                         start=(co == 0), stop=(co == 15))
                    sg2 = n2s.tile([1, R], f32)
                    nc.scalar.activation(sg2[:], ss2[:], Sqrt,
                                         bias=eps_t[0:1, :], scale=1.0 / C)
                    bc2 = psb2.tile([128, R], f32, tag="bc2")
                    nc.tensor.matmul(bc2[:], ones_row[:], sg2[:],
                                     start=True, stop=True)
                    b2sb = n2s.tile([128, R], f32)
                    nc.vector.reciprocal_approx_fast(out=b2sb[:], in_=bc2[:])
                    for co in range(16):
                        cs_ = slice(co * R, (co + 1) * R)
                        nc.vector.tensor_mul(xn2_sb[:, cs_], x2_sb[:, cs_], b2sb[:])

                # ================= phase 6a: MLP fc1/fc2 + swiglu =============
                with (
                    tc.tile_pool(name="w3s", bufs=3) as w3p,
                ):
                    w3_tiles = []
                    for co in range(16):
                        w3t = w3p.tile([128, NFB * 128], bf16, name=f"w3t{co}",
                                       tag="w3w")
                        nc.sync.dma_start(w3t[:], w3[co])
                        w3_tiles.append(w3t)
                    with (
                        nc.named_scope("mlp_fc"),
                        tc.tile_pool(name="hsil", bufs=2) as hsp,
                        tc.tile_pool(name="ps_h1", bufs=2, space="PSUM") as psh1,
                        tc.tile_pool(name="ps_h2", bufs=2, space="PSUM") as psh2,
                    ):
                        for fb in range(NFB):
                            wt = w12_tiles[fb]
                            h1 = psh1.tile([128, R], f32, tag="h1")
                            h2 = psh2.tile([128, R], f32, tag="h2")
                            for cb in range(16):
                                nc.tensor.matmul(
                                    h1[:], wt[:, cb * 128:(cb + 1) * 128],
                                    xn2_sb[:, cb * R:(cb + 1) * R],
                                    start=(cb == 0), stop=(cb == 15))
                            for cb in range(16):
                                nc.tensor.matmul(
                                    h2[:], wt[:, (16 + cb) * 128:(17 + cb) * 128],
                                    xn2_sb[:, cb * R:(cb + 1) * R],
                                    start=(cb == 0), stop=(cb == 15))
                            hs = hsp.tile([128, R], f32, tag="hs")
                            nc.scalar.activation(hs[:], h1[:], Silu)
                            nc.vector.tensor_mul(h_sb[:, fb * R:(fb + 1) * R],
                                                 hs[:], h2[:])

                    # ============== phase 6b: MLP proj + final residual =======
                    with (
                        nc.named_scope("mlp_proj"),
                        tc.tile_pool(name="outp", bufs=3) as outp,
                        tc.tile_pool(name="ps_o", bufs=2, space="PSUM") as pso,
                    ):
                        for co in range(16):
                            w3t = w3_tiles[co]
                            op = pso.tile([128, R], f32, tag="o")
                            for fb in range(NFB):
                                nc.tensor.matmul(
                                    op[:], w3t[:, fb * 128:(fb + 1) * 128],
                                    h_sb[:, fb * R:(fb + 1) * R],
                                    start=(fb == 0), stop=(fb == NFB - 1))
                            osb = outp.tile([128, R], f32, tag="osb")
                            nc.vector.tensor_add(osb[:], op[:],
                                                 x2_sb[:, co * R:(co + 1) * R])
                            nc.scalar.dma_start(outT[co * 128:(co + 1) * 128, :], osb[:])

    nc.compile()
    return nc


def _prep_inputs(inputs):
    """Host-side sharding / layout / dtype prep. Returns per-core in_maps."""
    x = np.asarray(inputs["x"], np.float32)[0]        # (T, C)
    cos = np.asarray(inputs["cos"], np.float32)[0]    # (T, HS)
    sin = np.asarray(inputs["sin"], np.float32)[0]
    qkv_w = np.asarray(inputs["qkv_w"], np.float32)   # (4096, C)
    proj_w = np.asarray(inputs["proj_w"], np.float32)  # (C, 2048)
    fc1_w = np.asarray(inputs["fc1_w"], np.float32)   # (FFN, C)
    fc2_w = np.asarray(inputs["fc2_w"], np.float32)
    mlp_proj_w = np.asarray(inputs["mlp_proj_w"], np.float32)  # (C, FFN)
    n1 = np.asarray(inputs["norm1_w"], np.float32)
    n2 = np.asarray(inputs["norm2_w"], np.float32)

    xT = np.ascontiguousarray(x.T)                    # (C, T)
    # xP[p, cb*T + t] = xT[cb*128+p, t]
    xP = np.ascontiguousarray(
        xT.reshape(NCB, 128, T).transpose(1, 0, 2).reshape(128, NCB * T)).astype(BF16)
    cosT = np.ascontiguousarray(cos.T).astype(BF16)
    sinT = np.ascontiguousarray(sin.T).astype(BF16)

    qkv_eff = (qkv_w * n1[None, :]).astype(BF16)      # fold norm1 weight
    # per-core d-major weight tiles
    wqkv_cores = []
    for i in range(N_CORES):
        dblocks = [
            qkv_eff[(2 * i) * HS:(2 * i + 1) * HS],       # q0
            qkv_eff[(2 * i + 1) * HS:(2 * i + 2) * HS],   # q1
            qkv_eff[NH * HS + i * HS: NH * HS + (i + 1) * HS],            # k
            qkv_eff[(NH + NKV) * HS + i * HS: (NH + NKV) * HS + (i + 1) * HS],  # v
        ]
        # tile (db, cb): lhsT[p, f] = W^T[cb*128+p, db*128+f] = W[db*128+f, cb*128+p]
        blocks = [dblocks[db].T.reshape(NCB, 128, 128) for db in range(4)]
        arr = np.stack(blocks, axis=0)              # (db, cb, p, f)
        wqkv_cores.append(np.ascontiguousarray(
            arr.transpose(2, 0, 1, 3).reshape(128, 64 * 128)))

    projT = proj_w.T.astype(BF16)                   # (ych, cout)
    # y_all channel-block order after the two per-head A2As: blocks 0-7 are the
    # even global heads (local head 0 of cores 0-7), 8-15 the odd ones.
    perm = [2 * g for g in range(8)] + [2 * g + 1 for g in range(8)]
    wproj = np.ascontiguousarray(
        projT.reshape(16, 128, 16, 128)[perm].transpose(2, 1, 0, 3)
        .reshape(16, 128, 16 * 128))

    w1T = (fc1_w * n2[None, :]).T.astype(BF16)      # (C, FFN)
    w2T = (fc2_w * n2[None, :]).T.astype(BF16)
    # w12[fb][p, (s*16+cb)*128+f] = wsT[cb*128+p, fb*128+f]
    a1 = w1T.reshape(NCB, 128, NFB, 128)            # (cb, p, fb, f)
    a2 = w2T.reshape(NCB, 128, NFB, 128)
    w12 = np.ascontiguousarray(
        np.stack([a1, a2], axis=0)                  # (s, cb, p, fb, f)
        .transpose(3, 2, 0, 1, 4)                   # (fb, p, s, cb, f)
        .reshape(NFB, 128, 2 * 16 * 128))
    mlpT = mlp_proj_w.T.astype(BF16)                # (FFN, C)
    w3 = np.ascontiguousarray(
        mlpT.reshape(NFB, 128, 16, 128).transpose(2, 1, 0, 3).reshape(16, 128, NFB * 128))

    in_maps = []
    for i in range(N_CORES):
        rows = slice(i * R, (i + 1) * R)
        xrT = xT[:, rows]                           # (C, R)
        xrows = np.ascontiguousarray(
            xrT.reshape(16, 128, R).transpose(1, 0, 2).reshape(128, 16 * R))
        in_maps.append({
            "xP": xP, "cosT": cosT, "sinT": sinT,
            "wqkv": wqkv_cores[i], "wproj": wproj,
            "w12": w12, "w3": w3, "xrows": xrows,
        })
    return in_maps


def _run(inputs, trace=False):
    from concourse import bass_utils
    if "nc" not in _CACHE:
        _CACHE["nc"] = _build()
    nc = _CACHE["nc"]
    in_maps = _prep_inputs(inputs)
    res = bass_utils.run_bass_kernel_spmd(
        nc, in_maps, core_ids=list(range(N_CORES)), trace=trace)
    outs = []
    for i in range(N_CORES):
        outs.append(res.results[i]["outT"].T)       # (R, C)
    full = np.concatenate(outs, axis=0)[None]       # (1, T, C)
    return np.ascontiguousarray(full.astype(np.float32)), res


def kernel(**inputs):
    out, _ = _run(inputs, trace=False)
    return out


# revision 27
# speedup vs baseline: 1.0232x; 1.0232x over previous
"""Trainium2 Bass kernel for a dense GQA transformer block (B=1, T=2048, C=2048,
16 q heads / 8 kv heads, hs=128, SwiGLU FFN=5632), SPMD across 8 NeuronCores.

Sharding: tensor-parallel attention (2 q heads + 1 kv head per core, full T),
one AllToAll per local head to re-shard from head-parallel to row-parallel,
then the attn projection, residual, norm2 and the whole MLP run row-parallel
(256 rows/core, full weights streamed from HBM as bf16).

Key scheduling ideas vs a straightforward version:
- Lazy rms-norm 1: qkv matmuls run on RAW x (so they start as soon as x is
  resident); the per-token 1/rms scale r is computed concurrently (squares on
  the Act engine, column sums via ones-matmuls) and folded into the RoPE
  cos/sin tables for q/k and into the PSUM->SBUF copy of v (tensor_scalar).
- Attention: softmax denominators accumulate on the PE via per-block
  ones-matmuls into PSUM (no DVE adds on the critical path); exp runs on Act;
  causal masking is a DVE multiply with a precomputed mask.
- The two per-head AllToAlls are hidden behind attention head 1 and behind a
  split attn-projection (first accumulate head-0's 8 y-blocks into 16 open
  PSUM co-tiles, then head-1's 8 blocks when its A2A lands).

All activations stay feature-major [C, T]/[HS, T]; matmuls map directly onto
the PE; partition-dim reductions/broadcasts use ones matmuls. bf16 inputs to
the PE with fp32 PSUM accumulation.
"""

import numpy as np
import ml_dtypes

N_CORES = 8
T = 2048
C = 2048
NH = 16
NKV = 8
HS = 128
FFN = 5632
EPS = 1e-5
R = T // N_CORES          # 256 rows (tokens) per core after the A2A
NCB = C // 128            # 16 feature blocks
NFB = FFN // 128          # 44 FFN blocks
NTCH = T // 512           # 4 T-chunks of 512
SM_SCALE = 1.0 / np.sqrt(np.float32(HS))
SM_BIAS = -10.0           # softmax exp bias; max |score| measured ~7, f32 exp safe
BF16 = ml_dtypes.bfloat16

_CACHE = {}


def _build():
    import concourse.mybir as mybir
    import concourse.tile as tile
    from concourse import bacc

    f32 = mybir.dt.float32
    bf16 = mybir.dt.bfloat16
    Exp = mybir.ActivationFunctionType.Exp
    Silu = mybir.ActivationFunctionType.Silu
    Sqrt = mybir.ActivationFunctionType.Sqrt
    Square = mybir.ActivationFunctionType.Square

    nc = bacc.Bacc(trn_type="TRN2", num_devices=N_CORES)

    # ---- kernel I/O (all host-pre-arranged to partition-major layouts) ----
    xP = nc.dram_tensor("xP", [128, NCB * T], bf16, kind="ExternalInput")
    cosT = nc.dram_tensor("cosT", [128, T], bf16, kind="ExternalInput")
    sinT = nc.dram_tensor("sinT", [128, T], bf16, kind="ExternalInput")
    # qkv weight tiles: [p, (db*16+cb)*128+f], db: 0=q0 1=q1 2=k 3=v
    wqkv = nc.dram_tensor("wqkv", [128, 64 * 128], bf16, kind="ExternalInput")
    # attn proj tiles per cout block: [co][p, yb*128+f]
    wproj = nc.dram_tensor("wproj", [16, 128, 16 * 128], bf16, kind="ExternalInput")
    # fc1|fc2 tiles per FFN block: [fb][p, (s*16+cb)*128+f]
    w12 = nc.dram_tensor("w12", [NFB, 128, 2 * 16 * 128], bf16, kind="ExternalInput")
    # mlp proj tiles per cout block: [co][p, fb*128+f]
    w3 = nc.dram_tensor("w3", [16, 128, NFB * 128], bf16, kind="ExternalInput")
    # residual x rows (this core's R tokens), c-major: [p, co*R+t]
    xrows = nc.dram_tensor("xrows", [128, 16 * R], f32, kind="ExternalInput")
    outT = nc.dram_tensor("outT", [C, R], f32, kind="ExternalOutput")

    with tile.TileContext(nc) as tc:
        with (
            tc.tile_pool(name="const", bufs=1) as constp,
            tc.tile_pool(name="dram", bufs=1, space="DRAM") as dramp,
            tc.tile_pool(name="w12s", bufs=7) as w12p,
            tc.tile_pool(name="qkv_acts", bufs=1) as qvp,
        ):
            # ---------------- constants ----------------
            ones_col = constp.tile([128, 1], bf16)
            nc.vector.memset(ones_col, 1.0)
            inv128_col = constp.tile([128, 1], bf16)
            nc.vector.memset(inv128_col, 1.0 / 128.0)
            ones_row = constp.tile([1, 128], f32)
            nc.vector.memset(ones_row, 1.0)
            eps_t = constp.tile([128, 1], f32)
            nc.vector.memset(eps_t, EPS)
            smbias_t = constp.tile([128, 1], f32)
            nc.vector.memset(smbias_t, SM_BIAS)
            masks = constp.tile([128, 4 * 512], bf16)
            nc.vector.memset(masks, 1.0)
            for j in range(4):
                # keep 1 where tq >= tk + 128*j  (iota = -x + y - 128j >= 0)
                nc.gpsimd.affine_select(
                    out=masks[:, j * 512:(j + 1) * 512],
                    in_=masks[:, j * 512:(j + 1) * 512],
                    compare_op=mybir.AluOpType.is_ge,
                    fill=0.0,
                    base=-128 * j,
                    pattern=[[1, 512]],
                    channel_multiplier=-1,
                )

            # a2a buffers (one collective per local head, fired as each
            # head's attention completes -> hides trigger latency + core skew)
            a2a_in0 = dramp.tile([8 * 128, R], bf16)
            a2a_out0 = dramp.tile([8 * 128, R], bf16)
            a2a_in1 = dramp.tile([8 * 128, R], bf16)
            a2a_out1 = dramp.tile([8 * 128, R], bf16)

            qk_sb = qvp.tile([128, 3 * T], bf16)     # roped+scaled q0|q1|k, d-major
            v_sb = qvp.tile([128, NCB * 128], bf16)  # scaled v token-major tiles

            with (
                tc.tile_pool(name="cs", bufs=1) as csp,
                tc.tile_pool(name="wqp", bufs=1) as wqpool,
                tc.tile_pool(name="xbfp", bufs=1) as xbfp,
            ):
                cs_sb = csp.tile([128, 2 * T], bf16)
                rcs = csp.tile([128, 2 * T], bf16)   # r-scaled cos|sin tables
                r_col = csp.tile([128, 16], f32)     # r as columns (v scaling)
                wq_sb = wqpool.tile([128, 64 * 128], bf16)
                xsb = xbfp.tile([128, NCB * T], bf16)

                # ===== phase 1+2: x load, lazy rms stats, qkv + rope =====
                # PE order: ss-sums (behind the x stream) -> q0 matmuls ->
                # norm-tail matmuls (bcast, r-columns) -> q1/k/v. The r chain
                # (Act sqrt -> PE bcast -> DVE recip) resolves while q0's
                # matmuls run, so the PE never waits on it.
                with (
                    nc.named_scope("qkv"),
                    tc.tile_pool(name="xsq", bufs=2) as sqp,
                    tc.tile_pool(name="n1small", bufs=1) as n1s,
                    tc.tile_pool(name="ps_ss", bufs=1, space="PSUM") as pss,
                    tc.tile_pool(name="ps_bc", bufs=1, space="PSUM") as psb,
                    tc.tile_pool(name="ropetmp", bufs=6) as rtp,
                    tc.tile_pool(name="ps_qk", bufs=4, space="PSUM") as psqk,
                    tc.tile_pool(name="ps_v", bufs=2, space="PSUM") as psv,
                ):
                    # chunk sum-rows packed into banks at partitions 0/32/64
                    # (96 = quadrant 3 is not allowed for matmul outputs); the
                    # 4th row borrows the bcast bank's ring slot.
                    ss_all = pss.tile([128, 512], f32, name="ssall", tag="ssall")
                    ss_b3 = psb.tile([128, 512], f32, name="ssb3", tag="bc")
                    ss_ps = [ss_all[0:1, :], ss_all[32:33, :],
                             ss_all[64:65, :], ss_b3[0:1, :]]
                    for cb in range(NCB):
                        xsl = xsb[:, cb * T:(cb + 1) * T]
                        nc.sync.dma_start(xsl, xP[:, cb * T:(cb + 1) * T])
                        xsq = sqp.tile([128, T], bf16, tag="xsq")
                        nc.vector.tensor_mul(xsq[:], xsl, xsl)
                        for t4 in range(NTCH):
                            nc.tensor.matmul(
                                ss_ps[t4], ones_col[:],
                                xsq[:, t4 * 512:(t4 + 1) * 512],
                                start=(cb == 0), stop=(cb == NCB - 1))
                    # qkv weights + rope tables load behind the x stream
                    for db in range(4):
                        nc.sync.dma_start(wq_sb[:, db * 2048:(db + 1) * 2048],
                                          wqkv[:, db * 2048:(db + 1) * 2048])
                    nc.scalar.dma_start(cs_sb[:, 0:T], cosT[:])
                    nc.scalar.dma_start(cs_sb[:, T:2 * T], sinT[:])

                    cosl = rcs[0:64, 0:T]
                    cosh = rcs[64:128, 0:T]
                    sinl = rcs[0:64, T:2 * T]
                    sinh = rcs[64:128, T:2 * T]

                    def qkv_mms(db, t4):
                        qp = psqk.tile([128, 512], f32, tag="qk",
                                       name=f"qk{db}_{t4}")
                        for cb in range(NCB):
                            nc.tensor.matmul(
                                qp[:],
                                wq_sb[:, (db * 16 + cb) * 128:
                                      (db * 16 + cb + 1) * 128],
                                xsb[:, cb * T + t4 * 512: cb * T + (t4 + 1) * 512],
                                start=(cb == 0), stop=(cb == NCB - 1))
                        return qp

                    def rope(db, t4, qp):
                        # SBUF-SBUF DVE ops need equal base partitions; PSUM
                        # inputs are exempt, so crossed-half terms read qp
                        # straight from PSUM.
                        ch = slice(t4 * 512, (t4 + 1) * 512)
                        dst = qk_sb[:, db * T + t4 * 512: db * T + (t4 + 1) * 512]
                        rc = rtp.tile([128, 512], bf16, tag="rc")
                        nc.vector.tensor_mul(rc[:], qp[:], rcs[:, 0:T][:, ch])
                        cross = rtp.tile([128, 512], bf16, tag="cross")
                        nc.vector.tensor_mul(cross[0:64, :], qp[64:128, :],
                                             sinl[:, ch])
                        nc.vector.tensor_mul(cross[64:128, :], qp[0:64, :],
                                             sinh[:, ch])
                        nc.vector.tensor_sub(dst[0:64, :], rc[0:64, :],
                                             cross[0:64, :])
                        nc.vector.tensor_add(dst[64:128, :], rc[64:128, :],
                                             cross[64:128, :])

                    # q0 matmuls issue first (they only need x)
                    q0ps = [qkv_mms(0, t4) for t4 in range(NTCH)]

                    # r = 1/sqrt(mean(x^2)+eps): sg=sqrt(...), bcast, recip
                    sg = n1s.tile([1, T], f32)
                    rbc = n1s.tile([128, T], bf16)
                    for t4 in range(NTCH):
                        ch = slice(t4 * 512, (t4 + 1) * 512)
                        nc.scalar.activation(sg[:, ch], ss_ps[t4],
                                             Sqrt, bias=eps_t[0:1, :], scale=1.0 / C)
                    for t4 in range(NTCH):
                        ch = slice(t4 * 512, (t4 + 1) * 512)
                        bc = psb.tile([128, 512], f32, tag="bc")
                        nc.tensor.matmul(bc[:], ones_row[:], sg[:, ch],
                                         start=True, stop=True)
                        rtmp = n1s.tile([128, 512], f32, tag="rtmp", bufs=1)
                        nc.vector.reciprocal_approx_fast(out=rtmp[:], in_=bc[:])
                        nc.vector.tensor_copy(rbc[:, ch], rtmp[:])
                    # r-scaled rope tables (fold norm into q/k path)
                    nc.vector.tensor_mul(rcs[:, 0:T], cs_sb[:, 0:T], rbc[:])
                    nc.vector.tensor_mul(rcs[:, T:2 * T], cs_sb[:, T:2 * T], rbc[:])
                    # r as token-major columns (for v scaling): transpose the
                    # broadcast tile via (rbc_tile)^T @ (ones/128)
                    rc_ps = pss.tile([128, 512], f32, name="rcol", tag="ssall")
                    for tb in range(16):
                        nc.tensor.matmul(rc_ps[:, tb:tb + 1],
                                         rbc[:, tb * 128:(tb + 1) * 128],
                                         inv128_col[:], start=True, stop=True)
                    nc.vector.tensor_copy(r_col[:], rc_ps[:, 0:16])

                    for t4 in range(NTCH):
                        rope(0, t4, q0ps[t4])
                    for db in (1, 2):
                        for t4 in range(NTCH):
                            rope(db, t4, qkv_mms(db, t4))
                    for tb_ in range(NCB):  # v token-major, r applied on copy-out
                        vp = psv.tile([128, 128], f32, tag="v")
                        for cb in range(NCB):
                            nc.tensor.matmul(
                                vp[:],
                                xsb[:, cb * T + tb_ * 128: cb * T + (tb_ + 1) * 128],
                                wq_sb[:, (48 + cb) * 128:(48 + cb + 1) * 128],
                                start=(cb == 0), stop=(cb == NCB - 1))
                        nc.vector.tensor_scalar_mul(
                            v_sb[:, tb_ * 128:(tb_ + 1) * 128], vp[:],
                            r_col[:, tb_:tb_ + 1])

            # x / cos / qkv-weight buffers freed here; MLP+proj weights stream in.
            # allocate all MLP fc weight tiles now so their DMAs start during attn
            w12_tiles = {}
            for fb in range(NFB):
                w12_tiles[fb] = w12p.tile(
                    [128, 2 * 16 * 128], bf16, name=f"w12t{fb}", tag="w12t")
                nc.sync.dma_start(w12_tiles[fb][:], w12[fb])

            with (
                tc.tile_pool(name="wprojs", bufs=6) as projp,
                tc.tile_pool(name="late", bufs=1) as latep,
            ):
                proj_tiles = []
                for co in range(16):
                    wt = projp.tile([128, 16 * 128], bf16, name=f"projw{co}",
                                    tag="projw")
                    nc.sync.dma_start(wt[:], wproj[co])
                    proj_tiles.append(wt)

                x2_sb = latep.tile([128, 16 * R], f32)
                xn2_sb = latep.tile([128, 16 * R], bf16)
                h_sb = latep.tile([128, NFB * R], bf16)
                y_all = latep.tile([128, 16 * R], bf16)

                # ============ phase 3: attention (2 heads per core) ============
                with (
                    nc.named_scope("attn"),
                    tc.tile_pool(name="pp_p", bufs=7) as ppool,
                    tc.tile_pool(name="pp_y", bufs=4) as ypool,
                    tc.tile_pool(name="attn_small", bufs=4) as asml,
                    tc.tile_pool(name="ps_s", bufs=4, space="PSUM") as ps_s,
                    tc.tile_pool(name="ps_y", bufs=2, space="PSUM") as ps_y,
                    tc.tile_pool(name="ps_sum", bufs=1, space="PSUM") as ps_sum,
                    tc.tile_pool(name="ps_abc", bufs=1, space="PSUM") as ps_abc,
                ):
                    for h in range(2):
                        a2a_in_h = a2a_in0 if h == 0 else a2a_in1
                        q_ap = qk_sb[:, h * T:(h + 1) * T]
                        k_ap = qk_sb[:, 2 * T:3 * T]
                        for qi in range(NTCH):
                            nkb = 4 * qi + 4
                            yp = ps_y.tile([128, 512], f32, tag="y")
                            sump = ps_sum.tile([1, 512], f32, tag="sum")
                            pend = []  # SW pipeline: AV + sum-MM trail scores
                            ptiles = {}

                            def flush(pair):
                                a, b = pair
                                # pair-sum on DVE (bf16), one sum-matmul per
                                # pair instead of per block
                                pa = ptiles.pop(a)
                                pb = ptiles.pop(b)
                                pr = asml.tile([128, 512], bf16, tag="pr")
                                nc.vector.tensor_add(pr[:], pa[:], pb[:])
                                nc.tensor.matmul(
                                    yp[:], v_sb[:, a * 128:(a + 1) * 128],
                                    pa[:], start=(a == 0), stop=False)
                                nc.tensor.matmul(
                                    yp[:], v_sb[:, b * 128:(b + 1) * 128],
                                    pb[:], start=False, stop=(b == nkb - 1))
                                nc.tensor.matmul(
                                    sump[:], ones_col[:], pr[:],
                                    start=(a == 0), stop=(b == nkb - 1))

                            for kb in range(nkb):
                                sp = ps_s.tile([128, 512], f32, tag="s")
                                nc.tensor.matmul(
                                    sp[:], k_ap[:, kb * 128:(kb + 1) * 128],
                                    q_ap[:, qi * 512:(qi + 1) * 512],
                                    start=True, stop=True)
                                pt = ppool.tile([128, 512], bf16, tag="p")
                                nc.scalar.activation(pt[:], sp[:], Exp,
                                                     bias=smbias_t[:],
                                                     scale=float(SM_SCALE))
                                if kb >= 4 * qi:
                                    moff = kb - 4 * qi
                                    nc.vector.tensor_mul(
                                        pt[:], pt[:],
                                        masks[:, moff * 512:(moff + 1) * 512])
                                ptiles[kb] = pt
                                if kb % 2 == 1:
                                    pend.append((kb - 1, kb))
                                if len(pend) > 1:
                                    flush(pend.pop(0))
                            while pend:
                                flush(pend.pop(0))
                            # 1/S broadcast, normalize, stage into A2A buffer
                            ssb = asml.tile([1, 512], f32, tag="ssb")
                            nc.scalar.copy(ssb[:], sump[:])
                            bcp = ps_abc.tile([128, 512], f32, tag="abc")
                            nc.tensor.matmul(bcp[:], ones_row[:], ssb[:],
                                             start=True, stop=True)
                            bsb = asml.tile([128, 512], f32, tag="bsb")
                            nc.vector.reciprocal_approx_fast(out=bsb[:], in_=bcp[:])
                            ysb = ypool.tile([128, 512], bf16, tag="ysb")
                            nc.vector.tensor_mul(ysb[:], yp[:], bsb[:])
                            # scatter two 256-token halves to this head's A2A
                            # buf. DVE's DMA queue: the sync queue is clogged
                            # with bulk weight streaming, which would delay the
                            # collective trigger by ~20us.
                            for half in range(2):
                                g = 2 * qi + half
                                nc.gpsimd.dma_start(
                                    a2a_in_h[128 * g: 128 * (g + 1), :],
                                    ysb[:, half * 256:(half + 1) * 256])
                        # fire this head's A2A as soon as its outputs are staged,
                        # and pull the result into SBUF immediately
                        a2a_out_h = a2a_out0 if h == 0 else a2a_out1
                        nc.gpsimd.collective_compute(
                            "AllToAll", mybir.AluOpType.bypass,
                            replica_groups=[list(range(N_CORES))],
                            ins=[a2a_in_h.opt()], outs=[a2a_out_h.opt()])
                        for g in range(8):
                            eng = (nc.gpsimd, nc.scalar)[g % 2]
                            eng.dma_start(
                                y_all[:, (h * 8 + g) * R:(h * 8 + g + 1) * R],
                                a2a_out_h[g * 128:(g + 1) * 128, :])

                # ==== phase 5: proj (split over the two A2As) + norm2 ====
                xq2s = []
                with (
                    nc.named_scope("proj"),
                    tc.tile_pool(name="xrow", bufs=1) as xrp,
                    tc.tile_pool(name="ps_acc", bufs=1, space="PSUM") as psa,
                ):
                    # 8 co accumulators = 8 PSUM banks (one accumulation group
                    # per bank: start=True clears has_written BANK-wide, so
                    # groups must never share a bank). The first 8 co split
                    # their yb accumulation across the two A2As; co 8-15 ring-
                    # reuse the banks afterwards.
                    xr_sb = xrp.tile([128, 16 * R], f32)
                    nc.gpsimd.dma_start(xr_sb[:], xrows[:])
                    aps = [psa.tile([128, R], f32, name=f"acc{j}", tag=f"acc{j}")
                           for j in range(8)]

                    def close_co(co, ap):
                        cs_ = slice(co * R, (co + 1) * R)
                        nc.vector.tensor_add(x2_sb[:, cs_], ap[:], xr_sb[:, cs_])
                        xq2 = latep.tile([128, R], bf16, name=f"xq2_{co}")
                        nc.vector.tensor_mul(xq2[:], x2_sb[:, cs_], x2_sb[:, cs_])
                        xq2s.append(xq2)

                    for yh in range(2):
                        for j in range(8):
                            wt = proj_tiles[j]
                            for yb in range(8 * yh, 8 * yh + 8):
                                nc.tensor.matmul(
                                    aps[j][:], wt[:, yb * 128:(yb + 1) * 128],
                                    y_all[:, yb * R:(yb + 1) * R],
                                    start=(yb == 0), stop=(yb == 15))
                    for j in range(8):
                        close_co(j, aps[j])
                    for j in range(8):
                        co = 8 + j
                        ap = psa.tile([128, R], f32, name=f"accb{j}", tag=f"acc{j}")
                        wt = proj_tiles[co]
                        for yb in range(16):
                            nc.tensor.matmul(
                                ap[:], wt[:, yb * 128:(yb + 1) * 128],
                                y_all[:, yb * R:(yb + 1) * R],
                                start=(yb == 0), stop=(yb == 15))
                        close_co(co, ap)

                with (
                    nc.named_scope("norm2"),
                    tc.tile_pool(name="n2small", bufs=1) as n2s,
                    tc.tile_pool(name="ps_ss2", bufs=1, space="PSUM") as pss2,
                    tc.tile_pool(name="ps_bc2", bufs=1, space="PSUM") as psb2,
                ):
                    ss2 = pss2.tile([1, R], f32, tag="ss2")
                    for co in range(16):
                        nc.tensor.matmul(ss2[:], ones_col[:], xq2s[co][:],
                                         start=(co == 0), stop=(co == 15))
                    sg2 = n2s.tile([1, R], f32)
                    nc.scalar.activation(sg2[:], ss2[:], Sqrt,
                                         bias=eps_t[0:1, :], scale=1.0 / C)
                    bc2 = psb2.tile([128, R], f32, tag="bc2")
                    nc.tensor.matmul(bc2[:], ones_row[:], sg2[:],
                                     start=True, stop=True)
                    b2sb = n2s.tile([128, R], f32)
                    nc.vector.reciprocal_approx_fast(out=b2sb[:], in_=bc2[:])
                    for co in range(16):
                        cs_ = slice(co * R, (co + 1) * R)
                        nc.vector.tensor_mul(xn2_sb[:, cs_], x2_sb[:, cs_], b2sb[:])

                # ================= phase 6a: MLP fc1/fc2 + swiglu =============
                with (
                    tc.tile_pool(name="w3s", bufs=3) as w3p,
                ):
                    w3_tiles = []
                    for co in range(16):
                        w3t = w3p.tile([128, NFB * 128], bf16, name=f"w3t{co}",
                                       tag="w3w")
                        nc.sync.dma_start(w3t[:], w3[co])
                        w3_tiles.append(w3t)
                    with (
                        nc.named_scope("mlp_fc"),
                        tc.tile_pool(name="hsil", bufs=2) as hsp,
                        tc.tile_pool(name="ps_h1", bufs=3, space="PSUM") as psh1,
                        tc.tile_pool(name="ps_h2", bufs=3, space="PSUM") as psh2,
                    ):
                        for fb in range(NFB):
                            wt = w12_tiles[fb]
                            h1 = psh1.tile([128, R], f32, tag="h1")
                            h2 = psh2.tile([128, R], f32, tag="h2")
                            for cb in range(16):
                                nc.tensor.matmul(
                                    h1[:], wt[:, cb * 128:(cb + 1) * 128],
                                    xn2_sb[:, cb * R:(cb + 1) * R],
                                    start=(cb == 0), stop=(cb == 15))
                            for cb in range(16):
                                nc.tensor.matmul(
                                    h2[:], wt[:, (16 + cb) * 128:(17 + cb) * 128],
                                    xn2_sb[:, cb * R:(cb + 1) * R],
                                    start=(cb == 0), stop=(cb == 15))
                            hs = hsp.tile([128, R], f32, tag="hs")
                            nc.scalar.activation(hs[:], h1[:], Silu)
                            nc.vector.tensor_mul(h_sb[:, fb * R:(fb + 1) * R],
                                                 hs[:], h2[:])

                    # ============== phase 6b: MLP proj + final residual =======
                    with (
                        nc.named_scope("mlp_proj"),
                        tc.tile_pool(name="outp", bufs=3) as outp,
                        tc.tile_pool(name="ps_o", bufs=3, space="PSUM") as pso,
                    ):
                        for co in range(16):
                            w3t = w3_tiles[co]
                            op = pso.tile([128, R], f32, tag="o")
                            for fb in range(NFB):
                                nc.tensor.matmul(
                                    op[:], w3t[:, fb * 128:(fb + 1) * 128],
                                    h_sb[:, fb * R:(fb + 1) * R],
                                    start=(fb == 0), stop=(fb == NFB - 1))
                            osb = outp.tile([128, R], f32, tag="osb")
                            nc.vector.tensor_add(osb[:], op[:],
                                                 x2_sb[:, co * R:(co + 1) * R])
                            nc.scalar.dma_start(outT[co * 128:(co + 1) * 128, :], osb[:])

    nc.compile()
    return nc


def _prep_inputs(inputs):
    """Host-side sharding / layout / dtype prep. Returns per-core in_maps."""
    x = np.asarray(inputs["x"], np.float32)[0]        # (T, C)
    cos = np.asarray(inputs["cos"], np.float32)[0]    # (T, HS)
    sin = np.asarray(inputs["sin"], np.float32)[0]
    qkv_w = np.asarray(inputs["qkv_w"], np.float32)   # (4096, C)
    proj_w = np.asarray(inputs["proj_w"], np.float32)  # (C, 2048)
    fc1_w = np.asarray(inputs["fc1_w"], np.float32)   # (FFN, C)
    fc2_w = np.asarray(inputs["fc2_w"], np.float32)
    mlp_proj_w = np.asarray(inputs["mlp_proj_w"], np.float32)  # (C, FFN)
    n1 = np.asarray(inputs["norm1_w"], np.float32)
    n2 = np.asarray(inputs["norm2_w"], np.float32)

    xT = np.ascontiguousarray(x.T)                    # (C, T)
    # xP[p, cb*T + t] = xT[cb*128+p, t]
    xP = np.ascontiguousarray(
        xT.reshape(NCB, 128, T).transpose(1, 0, 2).reshape(128, NCB * T)).astype(BF16)
    cosT = np.ascontiguousarray(cos.T).astype(BF16)
    sinT = np.ascontiguousarray(sin.T).astype(BF16)

    qkv_eff = (qkv_w * n1[None, :]).astype(BF16)      # fold norm1 weight
    # per-core d-major weight tiles
    wqkv_cores = []
    for i in range(N_CORES):
        dblocks = [
            qkv_eff[(2 * i) * HS:(2 * i + 1) * HS],       # q0
            qkv_eff[(2 * i + 1) * HS:(2 * i + 2) * HS],   # q1
            qkv_eff[NH * HS + i * HS: NH * HS + (i + 1) * HS],            # k
            qkv_eff[(NH + NKV) * HS + i * HS: (NH + NKV) * HS + (i + 1) * HS],  # v
        ]
        # tile (db, cb): lhsT[p, f] = W^T[cb*128+p, db*128+f] = W[db*128+f, cb*128+p]
        blocks = [dblocks[db].T.reshape(NCB, 128, 128) for db in range(4)]
        arr = np.stack(blocks, axis=0)              # (db, cb, p, f)
        wqkv_cores.append(np.ascontiguousarray(
            arr.transpose(2, 0, 1, 3).reshape(128, 64 * 128)))

    projT = proj_w.T.astype(BF16)                   # (ych, cout)
    # y_all channel-block order after the two per-head A2As: blocks 0-7 are the
    # even global heads (local head 0 of cores 0-7), 8-15 the odd ones.
    perm = [2 * g for g in range(8)] + [2 * g + 1 for g in range(8)]
    wproj = np.ascontiguousarray(
        projT.reshape(16, 128, 16, 128)[perm].transpose(2, 1, 0, 3)
        .reshape(16, 128, 16 * 128))

    w1T = (fc1_w * n2[None, :]).T.astype(BF16)      # (C, FFN)
    w2T = (fc2_w * n2[None, :]).T.astype(BF16)
    # w12[fb][p, (s*16+cb)*128+f] = wsT[cb*128+p, fb*128+f]
    a1 = w1T.reshape(NCB, 128, NFB, 128)            # (cb, p, fb, f)
    a2 = w2T.reshape(NCB, 128, NFB, 128)
    w12 = np.ascontiguousarray(
        np.stack([a1, a2], axis=0)                  # (s, cb, p, fb, f)
        .transpose(3, 2, 0, 1, 4)                   # (fb, p, s, cb, f)
        .reshape(NFB, 128, 2 * 16 * 128))
    mlpT = mlp_proj_w.T.astype(BF16)                # (FFN, C)
    w3 = np.ascontiguousarray(
        mlpT.reshape(NFB, 128, 16, 128).transpose(2, 1, 0, 3).reshape(16, 128, NFB * 128))

    in_maps = []
    for i in range(N_CORES):
        rows = slice(i * R, (i + 1) * R)
        xrT = xT[:, rows]                           # (C, R)
        xrows = np.ascontiguousarray(
            xrT.reshape(16, 128, R).transpose(1, 0, 2).reshape(128, 16 * R))
        in_maps.append({
            "xP": xP, "cosT": cosT, "sinT": sinT,
            "wqkv": wqkv_cores[i], "wproj": wproj,
            "w12": w12, "w3": w3, "xrows": xrows,
        })
    return in_maps


def _run(inputs, trace=False):
    from concourse import bass_utils
    if "nc" not in _CACHE:
        _CACHE["nc"] = _build()
    nc = _CACHE["nc"]
    in_maps = _prep_inputs(inputs)
    res = bass_utils.run_bass_kernel_spmd(
        nc, in_maps, core_ids=list(range(N_CORES)), trace=trace)
    outs = []
    for i in range(N_CORES):
        outs.append(res.results[i]["outT"].T)       # (R, C)
    full = np.concatenate(outs, axis=0)[None]       # (1, T, C)
    return np.ascontiguousarray(full.astype(np.float32)), res


def kernel(**inputs):
    out, _ = _run(inputs, trace=False)
    return out


# revision 31
# speedup vs baseline: 1.0452x; 1.0215x over previous
"""Trainium2 Bass kernel for a dense GQA transformer block (B=1, T=2048, C=2048,
16 q heads / 8 kv heads, hs=128, SwiGLU FFN=5632), SPMD across 8 NeuronCores.

Sharding: tensor-parallel attention (2 q heads + 1 kv head per core, full T),
one AllToAll per local head to re-shard from head-parallel to row-parallel,
then the attn projection, residual, norm2 and the whole MLP run row-parallel
(256 rows/core, full weights streamed from HBM as bf16).

Key scheduling ideas vs a straightforward version:
- Lazy rms-norm 1: qkv matmuls run on RAW x (so they start as soon as x is
  resident); the per-token 1/rms scale r is computed concurrently (squares on
  the Act engine, column sums via ones-matmuls) and folded into the RoPE
  cos/sin tables for q/k and into the PSUM->SBUF copy of v (tensor_scalar).
- Attention: softmax denominators accumulate on the PE via per-block
  ones-matmuls into PSUM (no DVE adds on the critical path); exp runs on Act;
  causal masking is a DVE multiply with a precomputed mask.
- The two per-head AllToAlls are hidden behind attention head 1 and behind a
  split attn-projection (first accumulate head-0's 8 y-blocks into 16 open
  PSUM co-tiles, then head-1's 8 blocks when its A2A lands).

All activations stay feature-major [C, T]/[HS, T]; matmuls map directly onto
the PE; partition-dim reductions/broadcasts use ones matmuls. bf16 inputs to
the PE with fp32 PSUM accumulation.
"""

import numpy as np
import ml_dtypes

N_CORES = 8
T = 2048
C = 2048
NH = 16
NKV = 8
HS = 128
FFN = 5632
EPS = 1e-5
R = T // N_CORES          # 256 rows (tokens) per core after the A2A
NCB = C // 128            # 16 feature blocks
NFB = FFN // 128          # 44 FFN blocks
NTCH = T // 512           # 4 T-chunks of 512
SM_SCALE = 1.0 / np.sqrt(np.float32(HS))
SM_BIAS = -10.0           # softmax exp bias; max |score| measured ~7, f32 exp safe
BF16 = ml_dtypes.bfloat16

_CACHE = {}


def _build():
    import concourse.mybir as mybir
    import concourse.tile as tile
    from concourse import bacc

    f32 = mybir.dt.float32
    bf16 = mybir.dt.bfloat16
    Exp = mybir.ActivationFunctionType.Exp
    Silu = mybir.ActivationFunctionType.Silu
    Sqrt = mybir.ActivationFunctionType.Sqrt
    Square = mybir.ActivationFunctionType.Square

    nc = bacc.Bacc(trn_type="TRN2", num_devices=N_CORES)

    # ---- kernel I/O (all host-pre-arranged to partition-major layouts) ----
    xP = nc.dram_tensor("xP", [128, NCB * T], bf16, kind="ExternalInput")
    cosT = nc.dram_tensor("cosT", [128, T], bf16, kind="ExternalInput")
    sinT = nc.dram_tensor("sinT", [128, T], bf16, kind="ExternalInput")
    # qkv weight tiles: [p, (db*16+cb)*128+f], db: 0=q0 1=q1 2=k 3=v
    wqkv = nc.dram_tensor("wqkv", [128, 64 * 128], bf16, kind="ExternalInput")
    # attn proj tiles per cout block: [co][p, yb*128+f]
    wproj = nc.dram_tensor("wproj", [16, 128, 16 * 128], bf16, kind="ExternalInput")
    # fc1|fc2 tiles per FFN block: [fb][p, (s*16+cb)*128+f]
    w12 = nc.dram_tensor("w12", [NFB, 128, 2 * 16 * 128], bf16, kind="ExternalInput")
    # mlp proj tiles per cout block: [co][p, fb*128+f]
    w3 = nc.dram_tensor("w3", [16, 128, NFB * 128], bf16, kind="ExternalInput")
    # residual x rows (this core's R tokens), c-major: [p, co*R+t]
    xrows = nc.dram_tensor("xrows", [128, 16 * R], f32, kind="ExternalInput")
    outT = nc.dram_tensor("outT", [C, R], f32, kind="ExternalOutput")

    with tile.TileContext(nc) as tc:
        with (
            tc.tile_pool(name="const", bufs=1) as constp,
            tc.tile_pool(name="dram", bufs=1, space="DRAM") as dramp,
            tc.tile_pool(name="w12s", bufs=7) as w12p,
            tc.tile_pool(name="qkv_acts", bufs=1) as qvp,
        ):
            # ---------------- constants ----------------
            ones_col = constp.tile([128, 1], bf16)
            nc.vector.memset(ones_col, 1.0)
            inv128_col = constp.tile([128, 1], bf16)
            nc.vector.memset(inv128_col, 1.0 / 128.0)
            ones_row = constp.tile([1, 128], f32)
            nc.vector.memset(ones_row, 1.0)
            ones_row_bf = constp.tile([1, 128], bf16)
            nc.vector.memset(ones_row_bf, 1.0)
            eps_t = constp.tile([128, 1], f32)
            nc.vector.memset(eps_t, EPS)
            smbias_t = constp.tile([128, 1], f32)
            nc.vector.memset(smbias_t, SM_BIAS)
            masks = constp.tile([128, 4 * 512], bf16)
            nc.vector.memset(masks, 1.0)
            for j in range(4):
                # keep 1 where tq >= tk + 128*j  (iota = -x + y - 128j >= 0)
                nc.gpsimd.affine_select(
                    out=masks[:, j * 512:(j + 1) * 512],
                    in_=masks[:, j * 512:(j + 1) * 512],
                    compare_op=mybir.AluOpType.is_ge,
                    fill=0.0,
                    base=-128 * j,
                    pattern=[[1, 512]],
                    channel_multiplier=-1,
                )

            # a2a buffers (one collective per local head, fired as each
            # head's attention completes -> hides trigger latency + core skew)
            a2a_in0 = dramp.tile([8 * 128, R], bf16)
            a2a_out0 = dramp.tile([8 * 128, R], bf16)
            a2a_in1 = dramp.tile([8 * 128, R], bf16)
            a2a_out1 = dramp.tile([8 * 128, R], bf16)

            qk_sb = qvp.tile([128, 3 * T], bf16)     # roped+scaled q0|q1|k, d-major
            v_sb = qvp.tile([128, NCB * 128], bf16)  # scaled v token-major tiles

            with (
                tc.tile_pool(name="cs", bufs=1) as csp,
                tc.tile_pool(name="wqp", bufs=1) as wqpool,
                tc.tile_pool(name="xbfp", bufs=1) as xbfp,
            ):
                cs_sb = csp.tile([128, 2 * T], bf16)
                rcs = csp.tile([128, 2 * T], bf16)   # r-scaled cos|sin tables
                r_col = csp.tile([128, 16], f32)     # r as columns (v scaling)
                wq_sb = wqpool.tile([128, 64 * 128], bf16)
                xsb = xbfp.tile([128, NCB * T], bf16)

                # ===== phase 1+2: x load, lazy rms stats, qkv + rope =====
                # PE order: ss-sums (behind the x stream) -> q0 matmuls ->
                # norm-tail matmuls (bcast, r-columns) -> q1/k/v. The r chain
                # (Act sqrt -> PE bcast -> DVE recip) resolves while q0's
                # matmuls run, so the PE never waits on it.
                with (
                    nc.named_scope("qkv"),
                    tc.tile_pool(name="xsq", bufs=2) as sqp,
                    tc.tile_pool(name="n1small", bufs=1) as n1s,
                    tc.tile_pool(name="ps_ss", bufs=1, space="PSUM") as pss,
                    tc.tile_pool(name="ps_bc", bufs=1, space="PSUM") as psb,
                    tc.tile_pool(name="ropetmp", bufs=6) as rtp,
                    tc.tile_pool(name="ps_qk", bufs=4, space="PSUM") as psqk,
                    tc.tile_pool(name="ps_v", bufs=2, space="PSUM") as psv,
                ):
                    # chunk sum-rows packed into banks at partitions 0/32/64
                    # (96 = quadrant 3 is not allowed for matmul outputs); the
                    # 4th row borrows the bcast bank's ring slot.
                    ss_all = pss.tile([128, 512], f32, name="ssall", tag="ssall")
                    ss_b3 = psb.tile([128, 512], f32, name="ssb3", tag="bc")
                    ss_ps = [ss_all[0:1, :], ss_all[32:33, :],
                             ss_all[64:65, :], ss_b3[0:1, :]]
                    for cb in range(NCB):
                        xsl = xsb[:, cb * T:(cb + 1) * T]
                        nc.sync.dma_start(xsl, xP[:, cb * T:(cb + 1) * T])
                        xsq = sqp.tile([128, T], bf16, tag="xsq")
                        nc.scalar.activation(xsq[:], xsl, Square)
                        for t4 in range(NTCH):
                            nc.tensor.matmul(
                                ss_ps[t4], ones_col[:],
                                xsq[:, t4 * 512:(t4 + 1) * 512],
                                start=(cb == 0), stop=(cb == NCB - 1))
                    # qkv weights + rope tables load behind the x stream
                    for db in range(4):
                        nc.sync.dma_start(wq_sb[:, db * 2048:(db + 1) * 2048],
                                          wqkv[:, db * 2048:(db + 1) * 2048])
                    nc.scalar.dma_start(cs_sb[:, 0:T], cosT[:])
                    nc.scalar.dma_start(cs_sb[:, T:2 * T], sinT[:])

                    cosl = rcs[0:64, 0:T]
                    cosh = rcs[64:128, 0:T]
                    sinl = rcs[0:64, T:2 * T]
                    sinh = rcs[64:128, T:2 * T]

                    def qkv_mms(db, t4):
                        qp = psqk.tile([128, 512], f32, tag="qk",
                                       name=f"qk{db}_{t4}")
                        for cb in range(NCB):
                            nc.tensor.matmul(
                                qp[:],
                                wq_sb[:, (db * 16 + cb) * 128:
                                      (db * 16 + cb + 1) * 128],
                                xsb[:, cb * T + t4 * 512: cb * T + (t4 + 1) * 512],
                                start=(cb == 0), stop=(cb == NCB - 1))
                        return qp

                    def rope(db, t4, qp):
                        # SBUF-SBUF DVE ops need equal base partitions; PSUM
                        # inputs are exempt, so crossed-half terms read qp
                        # straight from PSUM.
                        ch = slice(t4 * 512, (t4 + 1) * 512)
                        dst = qk_sb[:, db * T + t4 * 512: db * T + (t4 + 1) * 512]
                        rc = rtp.tile([128, 512], bf16, tag="rc")
                        nc.vector.tensor_mul(rc[:], qp[:], rcs[:, 0:T][:, ch])
                        cross = rtp.tile([128, 512], bf16, tag="cross")
                        nc.vector.tensor_mul(cross[0:64, :], qp[64:128, :],
                                             sinl[:, ch])
                        nc.vector.tensor_mul(cross[64:128, :], qp[0:64, :],
                                             sinh[:, ch])
                        nc.vector.tensor_sub(dst[0:64, :], rc[0:64, :],
                                             cross[0:64, :])
                        nc.vector.tensor_add(dst[64:128, :], rc[64:128, :],
                                             cross[64:128, :])

                    # q0 matmuls issue first (they only need x)
                    q0ps = [qkv_mms(0, t4) for t4 in range(NTCH)]

                    # r = 1/sqrt(mean(x^2)+eps): sg=sqrt(...), bcast, recip
                    sg = n1s.tile([1, T], f32)
                    rbc = n1s.tile([128, T], bf16)
                    for t4 in range(NTCH):
                        ch = slice(t4 * 512, (t4 + 1) * 512)
                        nc.scalar.activation(sg[:, ch], ss_ps[t4],
                                             Sqrt, bias=eps_t[0:1, :], scale=1.0 / C)
                    for t4 in range(NTCH):
                        ch = slice(t4 * 512, (t4 + 1) * 512)
                        bc = psb.tile([128, 512], f32, tag="bc")
                        nc.tensor.matmul(bc[:], ones_row[:], sg[:, ch],
                                         start=True, stop=True)
                        rtmp = n1s.tile([128, 512], f32, tag="rtmp", bufs=1)
                        nc.vector.reciprocal_approx_fast(out=rtmp[:], in_=bc[:])
                        nc.vector.tensor_copy(rbc[:, ch], rtmp[:])
                    # r-scaled rope tables (fold norm into q/k path)
                    nc.vector.tensor_mul(rcs[:, 0:T], cs_sb[:, 0:T], rbc[:])
                    nc.vector.tensor_mul(rcs[:, T:2 * T], cs_sb[:, T:2 * T], rbc[:])
                    # r as token-major columns (for v scaling): transpose the
                    # broadcast tile via (rbc_tile)^T @ (ones/128)
                    rc_ps = pss.tile([128, 512], f32, name="rcol", tag="ssall")
                    for tb in range(16):
                        nc.tensor.matmul(rc_ps[:, tb:tb + 1],
                                         rbc[:, tb * 128:(tb + 1) * 128],
                                         inv128_col[:], start=True, stop=True)
                    nc.vector.tensor_copy(r_col[:], rc_ps[:, 0:16])

                    for t4 in range(NTCH):
                        rope(0, t4, q0ps[t4])
                    for db in (1, 2):
                        for t4 in range(NTCH):
                            rope(db, t4, qkv_mms(db, t4))
                    for tb_ in range(NCB):  # v token-major, r applied on copy-out
                        vp = psv.tile([128, 128], f32, tag="v")
                        for cb in range(NCB):
                            nc.tensor.matmul(
                                vp[:],
                                xsb[:, cb * T + tb_ * 128: cb * T + (tb_ + 1) * 128],
                                wq_sb[:, (48 + cb) * 128:(48 + cb + 1) * 128],
                                start=(cb == 0), stop=(cb == NCB - 1))
                        nc.scalar.mul(v_sb[:, tb_ * 128:(tb_ + 1) * 128],
                                      vp[:], r_col[:, tb_:tb_ + 1])

            # x / cos / qkv-weight buffers freed here; MLP+proj weights stream in.
            # allocate all MLP fc weight tiles now so their DMAs start during attn
            w12_tiles = {}
            for fb in range(NFB):
                w12_tiles[fb] = w12p.tile(
                    [128, 2 * 16 * 128], bf16, name=f"w12t{fb}", tag="w12t")
                nc.sync.dma_start(w12_tiles[fb][:], w12[fb])

            with (
                tc.tile_pool(name="wprojs", bufs=6) as projp,
                tc.tile_pool(name="late", bufs=1) as latep,
            ):
                proj_tiles = []
                for co in range(16):
                    wt = projp.tile([128, 16 * 128], bf16, name=f"projw{co}",
                                    tag="projw")
                    nc.sync.dma_start(wt[:], wproj[co])
                    proj_tiles.append(wt)

                x2_sb = latep.tile([128, 16 * R], f32)
                xn2_sb = latep.tile([128, 16 * R], bf16)
                h_sb = latep.tile([128, NFB * R], bf16)
                y_all = latep.tile([128, 16 * R], bf16)

                # ============ phase 3: attention (2 heads per core) ============
                with (
                    nc.named_scope("attn"),
                    tc.tile_pool(name="pp_p", bufs=7) as ppool,
                    tc.tile_pool(name="pp_y", bufs=4) as ypool,
                    tc.tile_pool(name="attn_small", bufs=4) as asml,
                    tc.tile_pool(name="ps_s", bufs=4, space="PSUM") as ps_s,
                    tc.tile_pool(name="ps_y", bufs=2, space="PSUM") as ps_y,
                    tc.tile_pool(name="ps_sum", bufs=1, space="PSUM") as ps_sum,
                    tc.tile_pool(name="ps_abc", bufs=1, space="PSUM") as ps_abc,
                ):
                    for h in range(2):
                        a2a_in_h = a2a_in0 if h == 0 else a2a_in1
                        q_ap = qk_sb[:, h * T:(h + 1) * T]
                        k_ap = qk_sb[:, 2 * T:3 * T]
                        for qi in range(NTCH):
                            nkb = 4 * qi + 4
                            yp = ps_y.tile([128, 512], f32, tag="y")
                            sump = ps_sum.tile([1, 512], f32, tag="sum")
                            pend = []  # SW pipeline: AV + sum-MM trail scores
                            ptiles = {}
                            prs = []

                            def flush(pair):
                                a, b = pair
                                # pair-sum on DVE (bf16); quad-sum -> one
                                # sum-matmul per 4 blocks
                                pa = ptiles.pop(a)
                                pb = ptiles.pop(b)
                                pr = asml.tile([128, 512], bf16, tag="pr")
                                nc.vector.tensor_add(pr[:], pa[:], pb[:])
                                nc.tensor.matmul(
                                    yp[:], v_sb[:, a * 128:(a + 1) * 128],
                                    pa[:], start=(a == 0), stop=False)
                                nc.tensor.matmul(
                                    yp[:], v_sb[:, b * 128:(b + 1) * 128],
                                    pb[:], start=False, stop=(b == nkb - 1))
                                prs.append(pr)
                                if len(prs) == 2:
                                    p0, p1 = prs
                                    prs.clear()
                                    prr = asml.tile([128, 512], bf16, tag="prr")
                                    nc.vector.tensor_add(prr[:], p0[:], p1[:])
                                    nc.tensor.matmul(
                                        sump[:], ones_col[:], prr[:],
                                        start=(b == 3), stop=(b == nkb - 1))

                            for kb in range(nkb):
                                sp = ps_s.tile([128, 512], f32, tag="s")
                                nc.tensor.matmul(
                                    sp[:], k_ap[:, kb * 128:(kb + 1) * 128],
                                    q_ap[:, qi * 512:(qi + 1) * 512],
                                    start=True, stop=True)
                                pt = ppool.tile([128, 512], bf16, tag="p")
                                nc.scalar.activation(pt[:], sp[:], Exp,
                                                     bias=smbias_t[:],
                                                     scale=float(SM_SCALE))
                                if kb >= 4 * qi:
                                    moff = kb - 4 * qi
                                    nc.vector.tensor_mul(
                                        pt[:], pt[:],
                                        masks[:, moff * 512:(moff + 1) * 512])
                                ptiles[kb] = pt
                                if kb % 2 == 1:
                                    pend.append((kb - 1, kb))
                                if len(pend) > 1:
                                    flush(pend.pop(0))
                            while pend:
                                flush(pend.pop(0))
                            # 1/S broadcast, normalize, stage into A2A buffer
                            ssb = asml.tile([1, 512], bf16, tag="ssb")
                            nc.scalar.copy(ssb[:], sump[:])
                            bcp = ps_abc.tile([128, 512], f32, tag="abc")
                            nc.tensor.matmul(bcp[:], ones_row_bf[:], ssb[:],
                                             start=True, stop=True)
                            bsb = asml.tile([128, 512], f32, tag="bsb")
                            nc.vector.reciprocal_approx_fast(out=bsb[:], in_=bcp[:])
                            ysb = ypool.tile([128, 512], bf16, tag="ysb")
                            nc.vector.tensor_mul(ysb[:], yp[:], bsb[:])
                            # scatter two 256-token halves to this head's A2A
                            # buf. DVE's DMA queue: the sync queue is clogged
                            # with bulk weight streaming, which would delay the
                            # collective trigger by ~20us.
                            for half in range(2):
                                g = 2 * qi + half
                                nc.gpsimd.dma_start(
                                    a2a_in_h[128 * g: 128 * (g + 1), :],
                                    ysb[:, half * 256:(half + 1) * 256])
                        # fire this head's A2A as soon as its outputs are staged,
                        # and pull the result into SBUF immediately
                        a2a_out_h = a2a_out0 if h == 0 else a2a_out1
                        nc.gpsimd.collective_compute(
                            "AllToAll", mybir.AluOpType.bypass,
                            replica_groups=[list(range(N_CORES))],
                            ins=[a2a_in_h.opt()], outs=[a2a_out_h.opt()])
                    # pull both A2A results only AFTER both collectives are
                    # triggered: a DMA trigger blocks its sequencer on the
                    # collective-done semaphore, so pulls issued mid-attention
                    # would stall the exp stream / staging behind them.
                    for h in range(2):
                        a2a_out_h = a2a_out0 if h == 0 else a2a_out1
                        for g in range(8):
                            eng = (nc.gpsimd, nc.scalar)[g % 2]
                            eng.dma_start(
                                y_all[:, (h * 8 + g) * R:(h * 8 + g + 1) * R],
                                a2a_out_h[g * 128:(g + 1) * 128, :])

                # ==== phase 5: proj (split over the two A2As) + norm2 ====
                xq2s = []
                with (
                    nc.named_scope("proj"),
                    tc.tile_pool(name="xrow", bufs=1) as xrp,
                    tc.tile_pool(name="ps_acc", bufs=1, space="PSUM") as psa,
                ):
                    # 8 co accumulators = 8 PSUM banks (one accumulation group
                    # per bank: start=True clears has_written BANK-wide, so
                    # groups must never share a bank). The first 8 co split
                    # their yb accumulation across the two A2As; co 8-15 ring-
                    # reuse the banks afterwards.
                    xr_sb = xrp.tile([128, 16 * R], f32)
                    nc.gpsimd.dma_start(xr_sb[:], xrows[:])
                    aps = [psa.tile([128, R], f32, name=f"acc{j}", tag=f"acc{j}")
                           for j in range(8)]

                    def close_co(co, ap):
                        cs_ = slice(co * R, (co + 1) * R)
                        nc.vector.tensor_add(x2_sb[:, cs_], ap[:], xr_sb[:, cs_])
                        xq2 = latep.tile([128, R], bf16, name=f"xq2_{co}")
                        nc.vector.tensor_mul(xq2[:], x2_sb[:, cs_], x2_sb[:, cs_])
                        xq2s.append(xq2)

                    for yh in range(2):
                        for j in range(8):
                            wt = proj_tiles[j]
                            for yb in range(8 * yh, 8 * yh + 8):
                                nc.tensor.matmul(
                                    aps[j][:], wt[:, yb * 128:(yb + 1) * 128],
                                    y_all[:, yb * R:(yb + 1) * R],
                                    start=(yb == 0), stop=(yb == 15))
                    for j in range(8):
                        close_co(j, aps[j])
                    for j in range(8):
                        co = 8 + j
                        ap = psa.tile([128, R], f32, name=f"accb{j}", tag=f"acc{j}")
                        wt = proj_tiles[co]
                        for yb in range(16):
                            nc.tensor.matmul(
                                ap[:], wt[:, yb * 128:(yb + 1) * 128],
                                y_all[:, yb * R:(yb + 1) * R],
                                start=(yb == 0), stop=(yb == 15))
                        close_co(co, ap)

                with (
                    nc.named_scope("norm2"),
                    tc.tile_pool(name="n2small", bufs=1) as n2s,
                    tc.tile_pool(name="ps_ss2", bufs=1, space="PSUM") as pss2,
                    tc.tile_pool(name="ps_bc2", bufs=1, space="PSUM") as psb2,
                ):
                    ss2 = pss2.tile([1, R], f32, tag="ss2")
                    for co in range(16):
                        nc.tensor.matmul(ss2[:], ones_col[:], xq2s[co][:],
                                         start=(co == 0), stop=(co == 15))
                    sg2 = n2s.tile([1, R], f32)
                    nc.scalar.activation(sg2[:], ss2[:], Sqrt,
                                         bias=eps_t[0:1, :], scale=1.0 / C)
                    bc2 = psb2.tile([128, R], f32, tag="bc2")
                    nc.tensor.matmul(bc2[:], ones_row[:], sg2[:],
                                     start=True, stop=True)
                    b2sb = n2s.tile([128, R], f32)
                    nc.vector.reciprocal_approx_fast(out=b2sb[:], in_=bc2[:])
                    for co in range(16):
                        cs_ = slice(co * R, (co + 1) * R)
                        nc.vector.tensor_mul(xn2_sb[:, cs_], x2_sb[:, cs_], b2sb[:])

                # ================= phase 6a: MLP fc1/fc2 + swiglu =============
                with (
                    tc.tile_pool(name="w3s", bufs=3) as w3p,
                ):
                    w3_tiles = []
                    for co in range(16):
                        w3t = w3p.tile([128, NFB * 128], bf16, name=f"w3t{co}",
                                       tag="w3w")
                        nc.sync.dma_start(w3t[:], w3[co])
                        w3_tiles.append(w3t)
                    with (
                        nc.named_scope("mlp_fc"),
                        tc.tile_pool(name="hsil", bufs=2) as hsp,
                        tc.tile_pool(name="ps_h1", bufs=3, space="PSUM") as psh1,
                        tc.tile_pool(name="ps_h2", bufs=3, space="PSUM") as psh2,
                    ):
                        for fb in range(NFB):
                            wt = w12_tiles[fb]
                            h1 = psh1.tile([128, R], f32, tag="h1")
                            h2 = psh2.tile([128, R], f32, tag="h2")
                            for cb in range(16):
                                nc.tensor.matmul(
                                    h1[:], wt[:, cb * 128:(cb + 1) * 128],
                                    xn2_sb[:, cb * R:(cb + 1) * R],
                                    start=(cb == 0), stop=(cb == 15))
                            for cb in range(16):
                                nc.tensor.matmul(
                                    h2[:], wt[:, (16 + cb) * 128:(17 + cb) * 128],
                                    xn2_sb[:, cb * R:(cb + 1) * R],
                                    start=(cb == 0), stop=(cb == 15))
                            hs = hsp.tile([128, R], f32, tag="hs")
                            nc.scalar.activation(hs[:], h1[:], Silu)
                            nc.vector.tensor_mul(h_sb[:, fb * R:(fb + 1) * R],
                                                 hs[:], h2[:])

                    # ============== phase 6b: MLP proj + final residual =======
                    with (
                        nc.named_scope("mlp_proj"),
                        tc.tile_pool(name="outp", bufs=3) as outp,
                        tc.tile_pool(name="ps_o", bufs=3, space="PSUM") as pso,
                    ):
                        for co in range(16):
                            w3t = w3_tiles[co]
                            op = pso.tile([128, R], f32, tag="o")
                            for fb in range(NFB):
                                nc.tensor.matmul(
                                    op[:], w3t[:, fb * 128:(fb + 1) * 128],
                                    h_sb[:, fb * R:(fb + 1) * R],
                                    start=(fb == 0), stop=(fb == NFB - 1))
                            osb = outp.tile([128, R], f32, tag="osb")
                            nc.vector.tensor_add(osb[:], op[:],
                                                 x2_sb[:, co * R:(co + 1) * R])
                            nc.scalar.dma_start(outT[co * 128:(co + 1) * 128, :], osb[:])

    nc.compile()
    return nc


def _prep_inputs(inputs):
    """Host-side sharding / layout / dtype prep. Returns per-core in_maps."""
    x = np.asarray(inputs["x"], np.float32)[0]        # (T, C)
    cos = np.asarray(inputs["cos"], np.float32)[0]    # (T, HS)
    sin = np.asarray(inputs["sin"], np.float32)[0]
    qkv_w = np.asarray(inputs["qkv_w"], np.float32)   # (4096, C)
    proj_w = np.asarray(inputs["proj_w"], np.float32)  # (C, 2048)
    fc1_w = np.asarray(inputs["fc1_w"], np.float32)   # (FFN, C)
    fc2_w = np.asarray(inputs["fc2_w"], np.float32)
    mlp_proj_w = np.asarray(inputs["mlp_proj_w"], np.float32)  # (C, FFN)
    n1 = np.asarray(inputs["norm1_w"], np.float32)
    n2 = np.asarray(inputs["norm2_w"], np.float32)

    xT = np.ascontiguousarray(x.T)                    # (C, T)
    # xP[p, cb*T + t] = xT[cb*128+p, t]
    xP = np.ascontiguousarray(
        xT.reshape(NCB, 128, T).transpose(1, 0, 2).reshape(128, NCB * T)).astype(BF16)
    cosT = np.ascontiguousarray(cos.T).astype(BF16)
    sinT = np.ascontiguousarray(sin.T).astype(BF16)

    qkv_eff = (qkv_w * n1[None, :]).astype(BF16)      # fold norm1 weight
    # per-core d-major weight tiles
    wqkv_cores = []
    for i in range(N_CORES):
        dblocks = [
            qkv_eff[(2 * i) * HS:(2 * i + 1) * HS],       # q0
            qkv_eff[(2 * i + 1) * HS:(2 * i + 2) * HS],   # q1
            qkv_eff[NH * HS + i * HS: NH * HS + (i + 1) * HS],            # k
            qkv_eff[(NH + NKV) * HS + i * HS: (NH + NKV) * HS + (i + 1) * HS],  # v
        ]
        # tile (db, cb): lhsT[p, f] = W^T[cb*128+p, db*128+f] = W[db*128+f, cb*128+p]
        blocks = [dblocks[db].T.reshape(NCB, 128, 128) for db in range(4)]
        arr = np.stack(blocks, axis=0)              # (db, cb, p, f)
        wqkv_cores.append(np.ascontiguousarray(
            arr.transpose(2, 0, 1, 3).reshape(128, 64 * 128)))

    projT = proj_w.T.astype(BF16)                   # (ych, cout)
    # y_all channel-block order after the two per-head A2As: blocks 0-7 are the
    # even global heads (local head 0 of cores 0-7), 8-15 the odd ones.
    perm = [2 * g for g in range(8)] + [2 * g + 1 for g in range(8)]
    wproj = np.ascontiguousarray(
        projT.reshape(16, 128, 16, 128)[perm].transpose(2, 1, 0, 3)
        .reshape(16, 128, 16 * 128))

    w1T = (fc1_w * n2[None, :]).T.astype(BF16)      # (C, FFN)
    w2T = (fc2_w * n2[None, :]).T.astype(BF16)
    # w12[fb][p, (s*16+cb)*128+f] = wsT[cb*128+p, fb*128+f]
    a1 = w1T.reshape(NCB, 128, NFB, 128)            # (cb, p, fb, f)
    a2 = w2T.reshape(NCB, 128, NFB, 128)
    w12 = np.ascontiguousarray(
        np.stack([a1, a2], axis=0)                  # (s, cb, p, fb, f)
        .transpose(3, 2, 0, 1, 4)                   # (fb, p, s, cb, f)
        .reshape(NFB, 128, 2 * 16 * 128))
    mlpT = mlp_proj_w.T.astype(BF16)                # (FFN, C)
    w3 = np.ascontiguousarray(
        mlpT.reshape(NFB, 128, 16, 128).transpose(2, 1, 0, 3).reshape(16, 128, NFB * 128))

    in_maps = []
    for i in range(N_CORES):
        rows = slice(i * R, (i + 1) * R)
        xrT = xT[:, rows]                           # (C, R)
        xrows = np.ascontiguousarray(
            xrT.reshape(16, 128, R).transpose(1, 0, 2).reshape(128, 16 * R))
        in_maps.append({
            "xP": xP, "cosT": cosT, "sinT": sinT,
            "wqkv": wqkv_cores[i], "wproj": wproj,
            "w12": w12, "w3": w3, "xrows": xrows,
        })
    return in_maps


def _run(inputs, trace=False):
    from concourse import bass_utils
    if "nc" not in _CACHE:
        _CACHE["nc"] = _build()
    nc = _CACHE["nc"]
    in_maps = _prep_inputs(inputs)
    res = bass_utils.run_bass_kernel_spmd(
        nc, in_maps, core_ids=list(range(N_CORES)), trace=trace)
    outs = []
    for i in range(N_CORES):
        outs.append(res.results[i]["outT"].T)       # (R, C)
    full = np.concatenate(outs, axis=0)[None]       # (1, T, C)
    return np.ascontiguousarray(full.astype(np.float32)), res


def kernel(**inputs):
    out, _ = _run(inputs, trace=False)
    return out
